# revision 2
# baseline (speedup 1.0000x reference)
"""CVRP decoder kernel for 8 Trainium2 NeuronCores (pure batch data-parallel).

v2 of the pipelined kernel: same software-pipeline skeleton as v1 (qproj(b+1)
and input prefetch emitted one instance ahead, phase2(b-1) interleaved between
MHA1(b) and MHA2(b), deferred output store), with these changes:
  - v/v_s f32 tiles stage through HWDGE into 4-tile groups; ONE Pool
    tensor_tensor (+0) per group casts+restrides into the 32-stride vaug
    layout (ones column persistent), replacing 12 per-tile DVE casts.
  - S padded to 512 and M to 1024: uniform 128-row l-tiles; mask pads are
    memset to -1e9 once per slot, k tails zeroed once, so no tail cases.
  - k transposes write 4 tiles into one PSUM bank; ONE batched DVE copy
    per 4 tiles instead of 4.
  - all PE transposes take f32r-bitcast inputs (1.5 cycles/row vs 2.0).
  - the u *= mask01 multiplies are split between DVE (2x bf16 mode) and Pool
    (engine-level parallelism; Pool was ~9% busy in v1).
Engine budget per core (cost model): ACT ~211us (softmax exps, the bound),
PE ~190us, DVE ~175us, Pool ~155us, SP.SEQ ~125us, SWDGE ~130us.
"""

import re
from contextlib import ExitStack

import numpy as np

import concourse.bass as bass
import concourse.mybir as mybir
import concourse.tile as tile
from concourse.masks import make_identity

# ---------------------------------------------------------------- constants
B, N, M, S, E, H, D = 64, 256, 1000, 500, 128, 8, 16
SQRT_E = 11.313708498984761
CLIP = 10.0
NINF = -1.0e9
NCORES = 8
BLOC = B // NCORES  # 8 batch instances per core

MP, SP_ = 1024, 512      # padded l extents (uniform 128-row tiles)
NLT1, NLT2 = MP // 128, SP_ // 128

FP32 = mybir.dt.float32
F32R = mybir.dt.float32r
BF16 = mybir.dt.bfloat16
AF = mybir.ActivationFunctionType

MM_DT = F32R

# mask-mul engine assignment per pass: tile index -> Pool (else DVE)
POOL_TILES_M1 = set()
POOL_TILES_M2 = set()


def _r(ap):
    """view an fp32 AP as the matmul dtype (bitwise)"""
    if MM_DT is FP32:
        return ap
    return ap.bitcast(MM_DT)


# ------------------------------------------------- tile drain-split patch
# This walrus build rejects >1 sync-wait on a Drain ("Too many sync wait
# commands"), so split the kernel-tail global-clock waits onto single-wait
# NOPs preceding the drain.
def _patch_tile_drain():
    from bass_rust import ScopedClock, VectorClock

    def _drain_and_barrier(self, tick_clock, wait_clock):
        gc = tick_clock.global_clock
        vals = [int(x) for x in re.findall(r"\d+", repr(gc))]
        for proc, tick in enumerate(vals):
            if tick > 0:
                partial = VectorClock()
                partial.require_at_least(proc, tick)
                nop = self.nc.sync.nop(nofuse=True, hint="split_drain_wait")
                wait_clock.add_sem_waits(nop.ins, ScopedClock({None: partial}))
        self.nc.sync.drain()  # waits covered by the NOPs above
        self.nc.all_engine_barrier()
        assert self.sems is not None
        popped = self.nc._tile_sem_poison_stack.pop()
        assert popped is self._sem_poison
        self.nc.clear_and_free_semaphores(list(self.sems.allocated().values()))
        self.nc.all_engine_barrier()

    tile.TileContext._drain_and_barrier = _drain_and_barrier


_patch_tile_drain()


def _legalize_single_waits(nc):
    """This walrus build accepts at most ONE sync-wait per instruction; hoist
    extra waits onto single-wait NOP carriers placed just before, on the same
    engine (engines execute in order, so the gate is preserved)."""
    n_multi_upd = 0
    for f in nc.m.functions:
        for bb in f.blocks:
            out = []
            for inst in bb.instructions:
                si = inst.sync_info
                if si is not None and len(si.on_wait) > 1:
                    waits = list(si.on_wait)
                    si.on_wait = waits[-1:]
                    for w in waits[:-1]:
                        nop = mybir.InstNoOp(
                            name=nc.get_next_instruction_name(), ins=[], outs=[])
                        nop.engine = inst.engine
                        nop.sync_info = mybir.SyncInfo(on_wait=[w], on_update=[])
                        out.append(nop)
                if si is not None and len(si.on_update) > 1:
                    n_multi_upd += 1
                out.append(inst)
            bb.instructions = out
    if n_multi_upd:
        print(f"WARNING: {n_multi_upd} instructions with >1 sync updates")


def build_nc(legalize=True):
    nc = bass.Bass(trn_type="TRN2", target_bir_lowering=False, debug=False,
                   dynamic_dma_scratch_size=65536)

    # DRAM I/O (per-core shard)
    eln = nc.dram_tensor("eln", [BLOC, N, E], FP32, kind="ExternalInput").ap()
    load = nc.dram_tensor("load", [BLOC, N], FP32, kind="ExternalInput").ap()
    solm = nc.dram_tensor("solm", [BLOC, N, S], FP32, kind="ExternalInput").ap()
    ninf = nc.dram_tensor("ninf", [BLOC, N, M], FP32, kind="ExternalInput").ap()
    k_in = nc.dram_tensor("k", [BLOC, H, M, D], FP32, kind="ExternalInput").ap()
    v_in = nc.dram_tensor("v", [BLOC, H, M, D], FP32, kind="ExternalInput").ap()
    ks_in = nc.dram_tensor("k_s", [BLOC, H, S, D], FP32, kind="ExternalInput").ap()
    vs_in = nc.dram_tensor("v_s", [BLOC, H, S, D], FP32, kind="ExternalInput").ap()
    shk = nc.dram_tensor("shk", [BLOC, E, M], FP32, kind="ExternalInput").ap()
    wq = nc.dram_tensor("wq", [E, E + 1], FP32, kind="ExternalInput").ap()
    wc = nc.dram_tensor("wc", [E, E], FP32, kind="ExternalInput").ap()
    out = nc.dram_tensor("out", [BLOC, N, M], FP32, kind="ExternalOutput").ap()

    with ExitStack() as ctx:
        tc = ctx.enter_context(tile.TileContext(nc))
        build_kernel(ctx, tc, eln, load, solm, ninf, k_in, v_in, ks_in, vs_in,
                     shk, wq, wc, out)
    if legalize:
        _legalize_single_waits(nc)
    return nc


def build_kernel(ctx, tc, eln, load, solm, ninf, k_in, v_in, ks_in, vs_in,
                 shk, wq, wc, out):
    nc = tc.nc
    ctx.enter_context(nc.allow_low_precision("fp32r rounding for PE matmuls"))

    # pools
    singles = ctx.enter_context(tc.tile_pool(name="singles", bufs=1))
    sb_u = ctx.enter_context(tc.tile_pool(name="sb_u", bufs=3))
    sb_misc = ctx.enter_context(tc.tile_pool(name="sb_misc", bufs=2))
    sb_big = ctx.enter_context(tc.tile_pool(name="sb_big", bufs=2))
    sb_kt = ctx.enter_context(tc.tile_pool(name="sb_kt", bufs=2))
    sb_m = ctx.enter_context(tc.tile_pool(name="sb_m", bufs=1))
    sb_v32 = ctx.enter_context(tc.tile_pool(name="sb_v32", bufs=1))
    ps_score = ctx.enter_context(tc.tile_pool(name="ps_score", bufs=2, space="PSUM"))
    ps_att = ctx.enter_context(tc.tile_pool(name="ps_att", bufs=1, space="PSUM"))
    ps_mtp = ctx.enter_context(tc.tile_pool(name="ps_mtp", bufs=1, space="PSUM"))
    ps_kt = ctx.enter_context(tc.tile_pool(name="ps_kt", bufs=1, space="PSUM"))
    ps_phase = ctx.enter_context(tc.tile_pool(name="ps_phase", bufs=1, space="PSUM"))

    def mtp_ps():
        return ps_mtp.tile([128, 512], BF16, name="psm", tag="psm")

    def ktb_ps():
        return ps_kt.tile([128, 512], FP32, name="psk", tag="psk")

    def phase_ps():
        return ps_phase.tile([128, 512], FP32, name="psp", tag="psp")

    # ---------------- once-per-kernel prep ----------------
    ident = singles.tile([128, 128], FP32)
    make_identity(nc, ident)
    ident_r = singles.tile([128, 128], MM_DT, name="ident_r", tag="ident_r")
    nc.vector.tensor_copy(ident_r, ident)
    ident_bf = singles.tile([128, 128], BF16, name="ident_bf", tag="ident_bf")
    make_identity(nc, ident_bf)

    # persistent double-buffered input slots
    ninf_slots, sol_slots, kin1_slots, kin2_slots = [], [], [], []
    vaug1_slots, vaug2_slots = [], []
    for s in range(2):
        nin = singles.tile([128, 2, MP], BF16, name=f"ninf{s}", tag=f"ninf{s}")
        # pad cols [M:MP) stay -1e9 forever (DMA writes [0:M) only)
        nc.vector.memset(
            bass.AP(tensor=nin.tensor, offset=nin.offset + M,
                    ap=[[nin.ap[0][0], 128], [MP, 2], [1, MP - M]]), NINF)
        ninf_slots.append(nin)
        sol = singles.tile([128, 2, SP_], BF16, name=f"sol{s}", tag=f"sol{s}")
        nc.vector.memset(
            bass.AP(tensor=sol.tensor, offset=sol.offset + S,
                    ap=[[sol.ap[0][0], 128], [SP_, 2], [1, SP_ - S]]), NINF)
        sol_slots.append(sol)
        k1 = singles.tile([128, NLT1, 128], MM_DT, name=f"kin1_{s}", tag=f"kin1_{s}")
        kin1_slots.append(k1)
        k2 = singles.tile([128, NLT2, 128], MM_DT, name=f"kin2_{s}", tag=f"kin2_{s}")
        # k_s tail tile zeroed once; the per-instance DMA overwrites rows
        # [0:116) so rows [116:128) (l in [500:512)) stay 0 forever
        nc.vector.memset(k2[:, NLT2 - 1, :].bitcast(mybir.dt.int32), 0)
        kin2_slots.append(k2)
        for (tag, nlt, lst) in (("v1", NLT1, vaug1_slots), ("v2", NLT2, vaug2_slots)):
            va = singles.tile([128, nlt, H * 32], BF16, name=f"vaug_{tag}{s}",
                              tag=f"vaug_{tag}{s}")
            nc.gpsimd.memset(va, 0.0)
            ones_ap = bass.AP(tensor=va.tensor, offset=va.offset + D,
                              ap=[[va.ap[0][0], 128], [H * 32, nlt], [32, H]])
            nc.vector.memset(ones_ap, 1.0)
            lst.append(va)

    def prefetch_eln(b):
        eln_sb = sb_misc.tile([128, 2, 128], MM_DT, tag="eln_sb")
        nc.sync.dma_start(out=eln_sb, in_=_r(bass.AP(
            tensor=eln.tensor, offset=eln.offset + b * N * E,
            ap=[[E, 128], [128 * E, 2], [1, E]])))
        load_sb = sb_misc.tile([1, 256], MM_DT, tag="load_sb")
        nc.sync.dma_start(out=load_sb, in_=_r(bass.AP(
            tensor=load.tensor, offset=load.offset + b * N, ap=[[0, 1], [1, N]])))
        return (eln_sb, load_sb)

    def prefetch_bulk(b):
        """inputs whose slots have no outstanding phase2 readers: issue BEFORE
        mha1(b) so the qproj chain and first tiles of b+1 are never waiting"""
        sl = b % 2
        for (kd, kin, nlt, LT) in ((k_in, kin1_slots[sl], NLT1, M),
                                   (ks_in, kin2_slots[sl], NLT2, S)):
            for lt in range(nlt):
                L = min(128, LT - lt * 128)
                nc.sync.dma_start(out=kin[0:L, lt, :], in_=_r(bass.AP(
                    tensor=kd.tensor, offset=kd.offset + (b * H * LT + lt * 128) * D,
                    ap=[[D, L], [LT * D, H], [1, D]])))
        sol = sol_slots[sl]
        nc.gpsimd.dma_start(
            out=bass.AP(tensor=sol.tensor, offset=sol.offset,
                        ap=[[sol.ap[0][0], 128], [SP_, 2], [1, S]]),
            in_=bass.AP(tensor=solm.tensor, offset=solm.offset + b * N * S,
                        ap=[[S, 128], [128 * S, 2], [1, S]]))
        # v tiles: f32 HWDGE loads into staging; cast+restride into the
        # vaug layout happens on Pool (batched per 4-tile group)
        v32s = []
        for (vd, nlt, LT, tag) in ((v_in, NLT1, M, "v32a"),
                                   (vs_in, NLT2, S, "v32b")):
            ngr = (nlt + 3) // 4
            for g in range(ngr):
                ntl = min(4, nlt - 4 * g)
                v32 = sb_v32.tile([128, 4, 128], FP32, tag=f"{tag}{g}")
                for j in range(ntl):
                    lt = 4 * g + j
                    L = min(128, LT - lt * 128)
                    nc.sync.dma_start(out=v32[0:L, j, :], in_=bass.AP(
                        tensor=vd.tensor,
                        offset=vd.offset + (b * H * LT + lt * 128) * D,
                        ap=[[D, L], [LT * D, H], [1, D]]))
                v32s.append((v32, ntl))
        for (va, gidx) in ((vaug1_slots[sl], (0, 1)), (vaug2_slots[sl], (2,))):
            for gi, g in enumerate(gidx):
                v32, ntl = v32s[g]
                dstv = bass.AP(tensor=va.tensor,
                               offset=va.offset + gi * 4 * H * 32,
                               ap=[[va.ap[0][0], 128], [H * 32, ntl],
                                   [32, H], [1, D]])
                srcv = bass.AP(tensor=v32.tensor, offset=v32.offset,
                               ap=[[v32.ap[0][0], 128], [128, ntl],
                                   [16, H], [1, D]])
                zb = bass.AP(tensor=zcol.tensor, offset=zcol.offset,
                             ap=[[zcol.ap[0][0], 128], [0, ntl], [0, H], [0, D]])
                nc.gpsimd.tensor_add(dstv, srcv, zb)

    def prefetch_late(b):
        """ninf/shk: their slots are read by phase2(b-2)/(b-1) interleaves, so
        these loads must be emitted after that phase2"""
        sl = b % 2
        nin = ninf_slots[sl]
        nc.gpsimd.dma_start(
            out=bass.AP(tensor=nin.tensor, offset=nin.offset,
                        ap=[[nin.ap[0][0], 128], [MP, 2], [1, M]]),
            in_=bass.AP(tensor=ninf.tensor, offset=ninf.offset + b * N * M,
                        ap=[[M, 128], [128 * M, 2], [1, M]]))
        shk_sb = sb_big.tile([128, M], MM_DT, tag="shk_sb")
        nc.sync.dma_start(out=shk_sb, in_=_r(shk[b]))
        return shk_sb

    zcol = singles.tile([128, 1], FP32, name="zcol", tag="zcol")
    nc.vector.memset(zcol, 0.0)

    # Wq first (it gates qproj(0), which gates everything)
    wq_sb = singles.tile([E, E + 1], MM_DT)
    nc.sync.dma_start(out=wq_sb, in_=_r(wq))
    wq_lc = singles.tile([1, 128], MM_DT)
    nc.sync.dma_start(out=wq_lc, in_=_r(bass.AP(
        tensor=wq.tensor, offset=wq.offset + E, ap=[[0, 1], [E + 1, E]])))

    # instance 0: eln/load first — qproj(0) gates everything
    pf0 = prefetch_eln(0)

    # Wq^T in head-spread layout: pack p in {0,1}; head h=4p+i sits at
    # partition rows 32i+d. wqT[i_dim, hd] = transpose(Wq[:, :128]).
    wqT_ps = phase_ps()
    nc.tensor.transpose(_r(wqT_ps[:, 0:128]), wq_sb[:, 0:128], ident_r)
    wqT = singles.tile([128, 128], MM_DT)
    nc.vector.tensor_copy(wqT, wqT_ps[:, 0:128])

    # Wc^T row-permuted to the attention PSUM layout: half c holds heads
    # 4c+i at rows 32i+d (rows 32i+16..31 zero). Strided DMAs from wc;
    # not needed until phase2(0), so built late to keep startup short.
    wcT_c = []
    for c in range(2):
        t = singles.tile([128, 128], MM_DT, name=f"wcT_c{c}", tag=f"wcT_c{c}")
        nc.vector.memset(t.bitcast(mybir.dt.int32), 0)
        for i in range(4):
            srcp = bass.AP(tensor=wc.tensor, offset=wc.offset + 64 * c + 16 * i,
                           ap=[[1, 16], [E, 128]])
            nc.sync.dma_start(out=t[32 * i:32 * i + 16, :], in_=_r(srcp))
        wcT_c.append(t)

    # persistent block-diagonal q tiles (zero blocks never rewritten)
    qtz_slots = [singles.tile([128, H * 256], MM_DT, name=f"qtz{i}",
                              tag=f"qtz{i}") for i in range(2)]
    for t in qtz_slots:
        nc.gpsimd.memset(t.bitcast(mybir.dt.int32), 0)

    # ---------------- per batch instance (software-pipelined) ----------------

    def qproj(b, pf):
        eln_sb, load_sb = pf
        elnT_ps = phase_ps()
        for nt in range(2):
            nc.tensor.transpose(_r(elnT_ps[:, nt * 128:(nt + 1) * 128]),
                                eln_sb[:, nt, :], ident_r)
        elnT = sb_misc.tile([128, 256], MM_DT, tag="elnT")
        nc.vector.tensor_copy(elnT, elnT_ps[:, 0:256])

        qT_ps = phase_ps()
        nc.tensor.matmul(qT_ps[:, 0:256], wqT, elnT, start=True, stop=False)
        nc.tensor.matmul(qT_ps[:, 0:256], wq_lc, load_sb, start=False, stop=True)
        qT = sb_misc.tile([128, 256], FP32, tag="qT")
        nc.vector.tensor_copy(qT, qT_ps[:, 0:256])
        qtz = qtz_slots[b % 2]
        for h in range(H):
            nc.sync.dma_start(out=qtz[16 * h:16 * h + 16, h * 256:(h + 1) * 256],
                              in_=_r(qT[16 * h:16 * h + 16, :]))

    def mha_pass(b, kin, vaug, mask2, nlt, pool_tiles, tagp):
        """One masked-MHA pass over nlt uniform 128-row l-tiles. Returns the
        normalized per-head attention output (att layout [128, 512])."""
        qtz = qtz_slots[b % 2]
        att_all = ps_att.tile([128, 512], FP32, name="att", tag="att")
        att_ps = [att_all[:, 0:256], att_all[:, 256:512]]
        m01T = sb_m.tile([128, NLT1, 256], BF16, name=f"m01T_{tagp}",
                         tag="m01T")
        ktf_groups = {}

        def prep_kgroup(g):
            # batched k transpose: 4 tiles into one PSUM bank, one DVE copy
            if g * 4 >= nlt:
                return
            nkt = min(4, nlt - g * 4)
            ktb = ktb_ps()
            for j in range(nkt):
                nc.tensor.transpose(_r(ktb[:, j * 128:(j + 1) * 128]),
                                    kin[:, g * 4 + j, :], ident_r)
            ktf4 = sb_kt.tile([128, 4, 128], MM_DT, tag="ktf")
            nc.vector.tensor_copy(ktf4[:, 0:nkt, :], ktb[:, 0:nkt * 128])
            ktf_groups[g] = ktf4

        def prep_maskpair(pr):
            # mask transpose for a PAIR of l tiles + 0/1 conversion on DVE
            if pr * 2 >= nlt:
                return
            mtp = mtp_ps()
            for j in range(2):
                lj = (pr * 2 + j) * 128
                for nt in range(2):
                    nc.tensor.transpose(
                        mtp[:, j * 256 + nt * 128:j * 256 + (nt + 1) * 128],
                        mask2[:, nt, lj:lj + 128], ident_bf)
            nc.vector.tensor_scalar(m01T[:, pr * 2:pr * 2 + 2, :], mtp[:, 0:512],
                                    -0.5, None, mybir.AluOpType.is_ge)

        def att_mms(lt, u):
            # attention output (+ denominator row), accumulated over l tiles
            for h in range(H):
                nc.tensor.matmul(att_ps[h // 4][32 * (h % 4):32 * (h % 4) + 32, :],
                                 vaug[:, lt, h * 32:(h + 1) * 32],
                                 u[:, h, :],
                                 start=(lt == 0), stop=(lt == nlt - 1),
                                 tile_position=(0, 32 * (h % 4)),
                                 skip_group_check=True)

        # prologue: first k group + first mask pair
        prep_kgroup(0)
        prep_maskpair(0)
        u_prev = None
        for lt in range(nlt):
            # prepare the NEXT k group / mask pair one step ahead so the PE
            # queue head never waits on a just-in-time dependency
            if lt % 4 == 0:
                prep_kgroup(lt // 4 + 1)
            if lt % 2 == 0:
                prep_maskpair(lt // 2 + 1)

            # scores: K=128 against stacked kT; block-diagonal qtz per head
            u = sb_u.tile([128, H, 256], BF16, tag="u")
            for p in range(2):
                sc_ps = ps_score.tile([128, 1024], FP32, tag="sc_ps")
                for j in range(2):
                    nc.tensor.matmul(sc_ps[:, j * 512:(j + 1) * 512],
                                     ktf_groups[lt // 4][:, lt % 4, :],
                                     qtz[:, (4 * p + 2 * j) * 256:
                                         (4 * p + 2 * j + 2) * 256],
                                     start=True, stop=True)
                nc.scalar.activation(u[:, 4 * p:4 * p + 4, :], sc_ps[:, :],
                                     AF.Exp, scale=0.25)
            # mask (broadcast one tile across all 8 heads); DVE or Pool
            mslice = m01T[:, lt, :]
            mb = bass.AP(tensor=mslice.tensor, offset=mslice.offset,
                         ap=[mslice.ap[0], [0, H], [1, 256]])
            eng = nc.gpsimd if lt in pool_tiles else nc.vector
            eng.tensor_mul(u, u, mb)

            # att matmuls run one tile BEHIND (their mask-mul had a full tile
            # period to land, so PE's in-order queue never stalls on them)
            if u_prev is not None:
                att_mms(lt - 1, u_prev)
            u_prev = u
        att_mms(nlt - 1, u_prev)

        # normalize in att layout: the denominator row (local row 16 of each
        # 32-row head block) broadcast across its block by ONE stream_shuffle,
        # DVE reciprocal, Pool multiply
        attc = sb_misc.tile([128, 512], MM_DT, tag="attc")
        nc.vector.tensor_copy(attc, att_all)
        dshuf = sb_misc.tile([128, 512], MM_DT, tag="dshuf")
        nc.vector.stream_shuffle(dshuf.bitcast(FP32), attc.bitcast(FP32), [16] * 32)
        nc.vector.reciprocal(dshuf, dshuf)
        mhc = sb_misc.tile([128, 512], MM_DT, tag=f"mhc_{tagp}")
        nc.gpsimd.tensor_mul(mhc, attc, dshuf)
        return mhc

    def phase2(state):
        b, mhc1, mhc2, nin, shk_sb = state
        mh = sb_misc.tile([128, 512], MM_DT, tag="mh")
        nc.gpsimd.tensor_add(mh, mhc1, mhc2)

        # ---- combine (two halves accumulate over the full hd contraction)
        cmb_ps = phase_ps()
        for c in range(2):
            nc.tensor.matmul(cmb_ps[:, 0:256], wcT_c[c],
                             mh[:, c * 256:(c + 1) * 256],
                             start=(c == 0), stop=(c == 1))
        cmb = sb_misc.tile([128, 256], MM_DT, tag="cmb")
        nc.vector.tensor_copy(cmb, cmb_ps[:, 0:256])

        # ---- final scores, tanh, mask, softmax
        h2 = sb_big.tile([128, 2, M], FP32, tag="h2")
        for nt in range(2):
            t_sb = sb_v32.tile([128, M], FP32, tag="t_sb")
            for mt2 in range(2):
                fs_ps = phase_ps()
                nc.tensor.matmul(fs_ps[:, 0:500],
                                 cmb[:, nt * 128:(nt + 1) * 128],
                                 shk_sb[:, mt2 * 500:(mt2 + 1) * 500],
                                 start=True, stop=True)
                nc.scalar.activation(t_sb[:, mt2 * 500:(mt2 + 1) * 500],
                                     fs_ps[:, 0:500], AF.Tanh,
                                     scale=float(1.0 / SQRT_E))
            # logits/10 = tanh + mask/10 (any large negative works after exp)
            ninf_nt = bass.AP(tensor=nin.tensor, offset=nin.offset + nt * MP,
                              ap=[[nin.ap[0][0], 128], [1, M]])
            nc.gpsimd.tensor_add(t_sb, t_sb, ninf_nt)
            h_sb = h2[:, nt, :]
            rowsum = sb_misc.tile([128, 1], FP32, tag="rowsum")
            nc.scalar.activation(h_sb, t_sb, AF.Exp, scale=float(CLIP),
                                 accum_out=rowsum)
            rs_r = sb_misc.tile([128, 1], FP32, tag="rs_r")
            nc.vector.reciprocal(rs_r, rowsum)
            nc.vector.tensor_scalar_mul(h_sb, h_sb, rs_r)
        # store deferred to after qproj(b+1) (SP in-order queue head-of-line)
        dsto = bass.AP(tensor=out.tensor, offset=out.offset + b * N * M,
                       ap=[[M, 128], [128 * M, 2], [1, M]])
        return dsto, h2

    pf = pf0
    qproj(0, pf)
    prefetch_bulk(0)
    shk_cur = prefetch_late(0)
    state = None
    pending_out = None
    for b in range(BLOC):
        sl = b % 2
        if b + 1 < BLOC:
            pf = prefetch_eln(b + 1)
            prefetch_bulk(b + 1)
        mhc1 = mha_pass(b, kin1_slots[sl], vaug1_slots[sl], ninf_slots[sl],
                        NLT1, POOL_TILES_M1, "m1")
        if b + 1 < BLOC:
            qproj(b + 1, pf)
        if state is not None:
            pending_out = phase2(state)
        shk_next = prefetch_late(b + 1) if b + 1 < BLOC else None
        mhc2 = mha_pass(b, kin2_slots[sl], vaug2_slots[sl], sol_slots[sl],
                        NLT2, POOL_TILES_M2, "m2")
        state = (b, mhc1, mhc2, ninf_slots[sl], shk_cur)
        shk_cur = shk_next
        if pending_out is not None:
            nc.sync.dma_start(out=pending_out[0], in_=pending_out[1])
            pending_out = None
    dsto, h2 = phase2(state)
    nc.sync.dma_start(out=dsto, in_=h2)


# ------------------------------------------------------------- entry point
_NC_CACHE = None


def kernel(**inputs):
    global _NC_CACHE
    from concourse.bass_utils import run_bass_kernel_spmd

    if _NC_CACHE is None:
        _NC_CACHE = build_nc()
    nc = _NC_CACHE
    res = run_bass_kernel_spmd(nc, _in_maps(inputs), core_ids=list(range(NCORES)))
    return np.concatenate([res.results[c]["out"] for c in range(NCORES)], axis=0)


def _in_maps(inputs):
    arrs = {
        "eln": "encoded_last_node", "load": "load", "solm": "sols_mask_pomo",
        "ninf": "ninf_mask", "k": "k", "v": "v", "k_s": "k_s", "v_s": "v_s",
        "shk": "single_head_key", "wq": "Wq_last", "wc": "W_combine",
    }
    data = {n: np.ascontiguousarray(np.asarray(inputs[key], np.float32))
            for n, key in arrs.items()}
    in_maps = []
    for c in range(NCORES):
        s = slice(c * BLOC, (c + 1) * BLOC)
        in_maps.append({n: (a[s] if n not in ("wq", "wc") else a)
                        for n, a in data.items()})
    return in_maps


def bench(inputs, iters=6):
    """Measure per-launch hardware execution time.

    A single launch through the axon PJRT tunnel is dominated by a fixed
    ~70-90ms client<->terminal round trip (a trivial 2-DMA kernel measures
    the same wall time as this kernel), so single-launch wall clock says
    nothing about the device. Launches pipeline perfectly through the
    tunnel (K chained launches ~= 1 RTT + K * device_time), so the slope
    between two chain lengths cancels the RTT and yields the on-device
    time per launch. Device-side serialization is forced by donating
    launch i's output as launch i+1's output buffer (the kernel fully
    overwrites its output, so correctness is unaffected).
    """
    import time
    import jax
    import concourse.mybir as mb
    from concourse import bass2jax
    from jax.experimental.shard_map import shard_map
    from jax.sharding import Mesh, NamedSharding, PartitionSpec

    global _NC_CACHE
    if _NC_CACHE is None:
        _NC_CACHE = build_nc()
    nc = _NC_CACHE
    bass2jax.install_neuronx_cc_hook()

    partition_name = nc.partition_id_tensor.name if nc.partition_id_tensor else None
    in_names, out_names, out_avals, zero_outs = [], [], [], []
    for alloc in nc.m.functions[0].allocations:
        if not isinstance(alloc, mb.MemoryLocationSet):
            continue
        name = alloc.memorylocations[0].name
        if alloc.kind == "ExternalInput":
            if name != partition_name:
                in_names.append(name)
        elif alloc.kind == "ExternalOutput":
            shape = tuple(alloc.tensor_shape)
            dtype = mb.dt.np(alloc.dtype)
            out_names.append(name)
            out_avals.append(jax.core.ShapedArray(shape, dtype))
            zero_outs.append(np.zeros((NCORES * shape[0], *shape[1:]), dtype))
    n_params = len(in_names)
    n_outs = len(out_avals)
    all_names = in_names + out_names + ([partition_name] if partition_name else [])
    donate = tuple(range(n_params, n_params + n_outs))

    def _body(*args):
        operands = list(args)
        if partition_name is not None:
            operands.append(bass2jax.partition_id_tensor())
        return tuple(bass2jax._bass_exec_p.bind(
            *operands, out_avals=tuple(out_avals), in_names=tuple(all_names),
            out_names=tuple(out_names), lowering_input_output_aliases=(),
            sim_require_finite=True, sim_require_nnan=True, nc=nc))

    devices = jax.devices()[:NCORES]
    mesh = Mesh(np.asarray(devices), ("core",))
    sharded = jax.jit(
        shard_map(_body, mesh=mesh,
                  in_specs=(PartitionSpec("core"),) * (n_params + n_outs),
                  out_specs=(PartitionSpec("core"),) * n_outs, check_rep=False),
        donate_argnums=donate, keep_unused=True)

    in_maps = _in_maps(inputs)
    concat_in = [np.concatenate([np.asarray(in_maps[c][nm]) for c in range(NCORES)],
                                axis=0) for nm in in_names]
    sh = NamedSharding(mesh, PartitionSpec("core"))
    dev_in = [jax.device_put(a, sh) for a in concat_in]

    def chain(k, outs):
        t0 = time.perf_counter()
        for _ in range(k):
            outs = list(sharded(*dev_in, *outs))
        jax.block_until_ready(outs)
        return time.perf_counter() - t0, outs

    outs = [jax.device_put(z, sh) for z in zero_outs]
    _, outs = chain(2, outs)  # warmup (compile + HAM)
    # The tunnel RTT and terminal load fluctuate by tens of ms between runs,
    # so one two-point slope is unreliable. Take back-to-back (K_lo, K_hi)
    # pairs -- temporal locality gives both runs similar interference -- and
    # report the median of the per-pair slopes.
    k_lo, k_hi = 24, 120
    slopes = []
    for rep in range(6):
        t_lo, outs = chain(k_lo, outs)
        t_hi, outs = chain(k_hi, outs)
        s = (t_hi - t_lo) / (k_hi - k_lo)
        slopes.append(s)
        print(f"  rep{rep}: T({k_lo})={t_lo*1e3:.1f}ms T({k_hi})={t_hi*1e3:.1f}ms"
              f" -> {s*1e6:.0f} us/launch")
    med = float(np.median(slopes))
    print(f"  median device time: {med*1e6:.0f} us/launch")
    return int(med * 1e9)


if __name__ == "__main__":
    build_nc()
    print("build ok")
